# revision 1
# baseline (speedup 1.0000x reference)
"""LoFTR encoder layer (linear attention) on 8 Trainium2 NeuronCores.

Sharding: core c -> (n = c//2, L-half = c%2). Each core processes 4096 query
tokens; K/V state (KV = K^T V' over full S=8192) is computed replicated per
batch element. All matmuls bf16 (fp32 PSUM accumulate). Activations flow
feature-major; both LayerNorms run token-major (per-partition stats).
elu(x)+1 = exp(min(x,0)) + relu(x). rsqrt/recip via exp(-a*ln(v)) so the
whole kernel uses one ACT table set (natural_log_exp_and_others).
"""

import numpy as np
import ml_dtypes

import concourse.bass as bass
import concourse.bacc as bacc
import concourse.tile as tile
from concourse import mybir
from concourse.bass_utils import run_bass_kernel_spmd

F32 = mybir.dt.float32
BF16 = mybir.dt.bfloat16
AF = mybir.ActivationFunctionType

D_MODEL = 256
NHEAD = 8
HEAD_DIM = 32
LN_EPS = 1e-7
ATTN_EPS = 1e-6


def build_kernel(nc, TBLK, SBLK):
    """Emit the per-core program. TBLK = query-token 128-blocks (32 full),
    SBLK = source-token 128-blocks (64 full). Returns nothing; declares
    DRAM tensors by name."""
    C = D_MODEL
    NT = TBLK // 4   # q-tiles of 512 tokens
    NSG = SBLK // 4  # source groups of 512 rows

    # ---- DRAM I/O (per-core, host pre-shaped) ----
    x_pre = nc.dram_tensor("x_pre", [128, 2, TBLK * 128], BF16,
                           kind="ExternalInput").ap()
    s_pre = nc.dram_tensor("s_pre", [128, 2, SBLK * 128], BF16,
                           kind="ExternalInput").ap()
    wq_t = nc.dram_tensor("wq_t", [128, 2, 2, 128], BF16, kind="ExternalInput").ap()
    wk_r = nc.dram_tensor("wk_r", [128, 2, 256], BF16, kind="ExternalInput").ap()
    wv_r = nc.dram_tensor("wv_r", [128, 2, 256], BF16, kind="ExternalInput").ap()
    wm_r = nc.dram_tensor("wm_r", [128, 2, 256], BF16, kind="ExternalInput").ap()
    w1_t = nc.dram_tensor("w1_t", [128, 4, 4, 128], BF16, kind="ExternalInput").ap()
    w2_r = nc.dram_tensor("w2_r", [128, 4, 256], BF16, kind="ExternalInput").ap()
    b1p_d = nc.dram_tensor("b1p", [128, 4], F32, kind="ExternalInput").ap()
    g2rep_d = nc.dram_tensor("g2rep", [128, 256], F32, kind="ExternalInput").ap()
    ident_d = nc.dram_tensor("ident", [128, 128], BF16, kind="ExternalInput").ap()
    res_d = nc.dram_tensor("res", [128, TBLK, C], F32, kind="ExternalOutput").ap()
    z_dram = nc.dram_tensor("z_dram", [NT, 8, 512], BF16).ap()

    from contextlib import ExitStack
    tc = nc.tc  # TileContext stored by caller
    ctx = ExitStack()
    nc._pool_ctx = ctx

    consts = ctx.enter_context(tc.tile_pool(name="consts", bufs=1))
    persist = ctx.enter_context(tc.tile_pool(name="persist", bufs=1))
    work = ctx.enter_context(tc.tile_pool(name="work", bufs=2))
    psA_cm = tc.tile_pool(name="psA", bufs=1, space="PSUM")
    psA = psA_cm.__enter__()

    # ---- constants ----
    wq = consts.tile([128, 2, 2, 128], BF16, name="wq")
    wk = consts.tile([128, 2, 256], BF16, name="wk")
    wv = consts.tile([128, 2, 256], BF16, name="wv")
    wm = consts.tile([128, 2, 256], BF16, name="wm")
    w1 = consts.tile([128, 4, 4, 128], BF16, name="w1")
    w2 = consts.tile([128, 4, 256], BF16, name="w2")
    b1p = consts.tile([128, 4], F32, name="b1p")
    g2rep = consts.tile([128, 256], F32, name="g2rep")
    eps_a = consts.tile([128, 1], F32, name="eps_a")
    eps_l = consts.tile([128, 1], F32, name="eps_l")
    ident = consts.tile([128, 128], BF16, name="ident")

    # ---- persistent activations ----
    xf = persist.tile([128, 2, TBLK * 128], BF16, name="xf")      # x feature-major
    qe = persist.tile([128, 2, TBLK * 128], BF16, name="qe")      # elu(q)+1 fm
    srcf = persist.tile([128, 2, SBLK * 128], BF16, name="srcf")  # source fm

    # inputs arrive feature-major from host; plain bulk DMAs
    nc.gpsimd.dma_start(out=xf[:], in_=x_pre)
    nc.gpsimd.dma_start(out=srcf[:], in_=s_pre)

    for dst, src in ((wq, wq_t), (wk, wk_r), (wv, wv_r), (wm, wm_r),
                     (w1, w1_t), (w2, w2_r), (b1p, b1p_d), (g2rep, g2rep_d),
                     (ident, ident_d)):
        nc.gpsimd.dma_start(out=dst[:], in_=src)
    nc.vector.memset(eps_a, ATTN_EPS)
    nc.vector.memset(eps_l, LN_EPS)

    # ---- phase 1a: Q linear + elu, all tiles ----
    for t in range(NT):
        cols = slice(512 * t, 512 * (t + 1))
        q_ps = psA.tile([128, 2, 512], F32, name="q_ps")
        for m in range(2):
            for k in range(2):
                nc.tensor.matmul(q_ps[:, m, :], lhsT=wq[:, k, m, :],
                                 rhs=xf[:, k, cols], start=(k == 0), stop=(k == 1))
        mneg = work.tile([128, 2, 512], BF16, name="mneg")
        nc.vector.tensor_scalar_min(mneg[:], q_ps[:], 0.0)   # min(q, 0)
        ee = work.tile([128, 2, 512], BF16, name="ee")
        nc.scalar.activation(out=ee[:], in_=mneg[:], func=AF.Exp, scale=1.0)
        nc.vector.scalar_tensor_tensor(
            out=qe[:, :, 512 * t:512 * (t + 1)], in0=q_ps[:],
            scalar=0.0, in1=ee[:],
            op0=mybir.AluOpType.max, op1=mybir.AluOpType.add)

    # ---- phase 1b: K/V + KV state over full S ----
    kv_ps = psA.tile([128, 2, 512], F32, name="kv_ps")
    for g in range(NSG):
        k_ps = psA.tile([128, 4, 256], F32, name="k_ps")
        v_ps = psA.tile([128, 4, 256], F32, name="v_ps")
        for j in range(4):
            scols = slice(512 * g + 128 * j, 512 * g + 128 * (j + 1))
            for k in range(2):
                nc.tensor.matmul(k_ps[:, j, :], lhsT=srcf[:, k, scols],
                                 rhs=wk[:, k, :], start=(k == 0), stop=(k == 1))
            for k in range(2):
                nc.tensor.matmul(v_ps[:, j, :], lhsT=srcf[:, k, scols],
                                 rhs=wv[:, k, :], start=(k == 0), stop=(k == 1))
        kmneg = work.tile([128, 4, 256], BF16, name="kmneg")
        nc.vector.tensor_scalar_min(kmneg[:], k_ps[:], 0.0)
        kee = work.tile([128, 4, 256], BF16, name="kee")
        nc.scalar.activation(out=kee[:], in_=kmneg[:], func=AF.Exp, scale=1.0)
        ke = work.tile([128, 4, 256], BF16, name="ke")
        nc.vector.scalar_tensor_tensor(
            out=ke[:], in0=k_ps[:], scalar=0.0, in1=kee[:],
            op0=mybir.AluOpType.max, op1=mybir.AluOpType.add)
        ve = work.tile([128, 4, 264], BF16, name="ve")
        nc.scalar.activation(out=ve[:, :, 0:256], in_=v_ps[:], func=AF.Copy)
        nc.vector.memset(ve[:, :, 256:257], 1.0)
        for j in range(4):
            for c in range(2):
                nc.tensor.matmul(
                    kv_ps[:, c, 0:257],
                    lhsT=ke[:, j, 128 * c:128 * (c + 1)],
                    rhs=ve[:, j, 0:257],
                    start=(g == 0 and j == 0), stop=(g == NSG - 1 and j == 3))

    # ---- BD (block-diag KV) + KsumBD ----
    bd = consts.tile([128, 2, 128], BF16, name="bd")
    ksumbd = consts.tile([128, 2, 8], BF16, name="ksumbd")
    nc.vector.memset(bd[:], 0.0)
    nc.vector.memset(ksumbd[:], 0.0)
    for c in range(2):
        for r in range(4):
            h = 4 * c + r
            rows = slice(32 * r, 32 * (r + 1))
            nc.scalar.activation(out=bd[rows, c, rows],
                                 in_=kv_ps[rows, c, 32 * h:32 * h + 32], func=AF.Copy)
            nc.scalar.activation(out=ksumbd[rows, c, h:h + 1],
                                 in_=kv_ps[rows, c, 256:257], func=AF.Copy)

    psA_cm.__exit__(None, None, None)
    psB_cm = tc.tile_pool(name="psB", bufs=1, space="PSUM")
    psB = psB_cm.__enter__()
    ctx.callback(lambda: psB_cm.__exit__(None, None, None))

    # ---- phase 2: per q-tile ----
    for t in range(NT):
        cols = slice(512 * t, 512 * (t + 1))
        # QdotK -> Z = 1/(. + eps) via exp(-ln)
        qk_ps = psB.tile([8, 512], F32, name="qk_ps", bufs=2)
        for c in range(2):
            nc.tensor.matmul(qk_ps[:], lhsT=ksumbd[:, c, :], rhs=qe[:, c, cols],
                             start=(c == 0), stop=(c == 1))
        zln = work.tile([8, 512], F32, name="zln")
        nc.scalar.activation(out=zln[:], in_=qk_ps[:], func=AF.Ln,
                             bias=eps_a[0:8, :], scale=1.0)
        z_sb = work.tile([8, 512], BF16, name="z_sb")
        nc.scalar.activation(out=z_sb[:], in_=zln[:], func=AF.Exp, scale=-1.0)
        nc.gpsimd.dma_start(out=z_dram[t], in_=z_sb[:])
        zrep = work.tile([128, 2, 512], BF16, name="zrep")
        for c in range(2):
            src_b = bass.AP(tensor=z_dram.tensor,
                            offset=t * 8 * 512 + 4 * c * 512,
                            ap=[[512, 4], [0, 32], [1, 512]])
            nc.gpsimd.dma_start(out=zrep[:, c, :], in_=src_b)
        # attention: BD matmul then Z multiply
        attn_ps = psB.tile([128, 2, 512], F32, name="attn_ps")
        for c in range(2):
            nc.tensor.matmul(attn_ps[:, c, :], lhsT=bd[:, c, :], rhs=qe[:, c, cols],
                             start=True, stop=True)
        attnz = work.tile([128, 2, 512], BF16, name="attnz")
        nc.vector.tensor_mul(attnz[:], attn_ps[:], zrep[:])
        # merge (token-major out)
        msg_ps = psB.tile([128, 4, 256], F32, name="msgx_ps")
        for j in range(4):
            for k in range(2):
                nc.tensor.matmul(msg_ps[:, j, :],
                                 lhsT=attnz[:, k, 128 * j:128 * (j + 1)],
                                 rhs=wm[:, k, :], start=(k == 0), stop=(k == 1))
        # LN1 (no g/b: folded into W1/b1')
        st1 = work.tile([128, 4, 6], F32, name="st1")
        mv1 = work.tile([128, 4, 2], F32, name="mv1")
        for j in range(4):
            nc.vector.bn_stats(out=st1[:, j, :], in_=msg_ps[:, j, :])
            nc.vector.bn_aggr(out=mv1[:, j, :], in_=st1[:, j, :])
        lnv1 = work.tile([128, 4], F32, name="lnv1")
        nc.scalar.activation(out=lnv1[:], in_=mv1[:, :, 1], func=AF.Ln,
                             bias=eps_l[:], scale=1.0)
        rstd1 = work.tile([128, 4], F32, name="rstd1")
        nc.scalar.activation(out=rstd1[:], in_=lnv1[:], func=AF.Exp, scale=-0.5)
        lnm = work.tile([128, 4, 256], BF16, name="lnm")
        for j in range(4):
            nc.vector.tensor_scalar(
                out=lnm[:, j, :], in0=msg_ps[:, j, :],
                scalar1=mv1[:, j, 0:1], scalar2=rstd1[:, j:j + 1],
                op0=mybir.AluOpType.subtract, op1=mybir.AluOpType.mult)
        lnT_ps = psB.tile([128, 2, 512], BF16, name="attn_ps")
        for j in range(4):
            for c in range(2):
                nc.tensor.transpose(out=lnT_ps[:, c, 128 * j:128 * (j + 1)],
                                    in_=lnm[:, j, 128 * c:128 * (c + 1)],
                                    identity=ident[:])
        lnmT = work.tile([128, 2, 512], BF16, name="lnmT")
        for c in range(2):
            nc.scalar.activation(out=lnmT[:, c, :], in_=lnT_ps[:, c, :],
                                 func=AF.Copy)
        # MLP1 (feature-major out) + relu(+bias), two PSUM halves
        h_sb = work.tile([128, 4, 512], BF16, name="h_sb")
        for half in range(2):
            h_ps = psB.tile([128, 2, 512], F32, name="h_ps")
            for mi in range(2):
                m = 2 * half + mi
                for k in range(4):
                    rhs = xf[:, k, cols] if k < 2 else lnmT[:, k - 2, :]
                    nc.tensor.matmul(h_ps[:, mi, :], lhsT=w1[:, k, m, :], rhs=rhs,
                                     start=(k == 0), stop=(k == 3))
            for mi in range(2):
                m = 2 * half + mi
                nc.scalar.activation(out=h_sb[:, m, :], in_=h_ps[:, mi, :],
                                     func=AF.Relu, bias=b1p[:, m:m + 1], scale=1.0)
        # MLP2 (token-major out)
        msg2_ps = psB.tile([128, 4, 256], F32, name="msgx_ps")
        for j in range(4):
            for k in range(4):
                nc.tensor.matmul(msg2_ps[:, j, :],
                                 lhsT=h_sb[:, k, 128 * j:128 * (j + 1)],
                                 rhs=w2[:, k, :], start=(k == 0), stop=(k == 3))
        # LN2 -> g2 * norm2 (residual + b2 added on host)
        st2 = work.tile([128, 4, 6], F32, name="st2")
        mv2 = work.tile([128, 4, 2], F32, name="mv2")
        for j in range(4):
            nc.vector.bn_stats(out=st2[:, j, :], in_=msg2_ps[:, j, :])
            nc.vector.bn_aggr(out=mv2[:, j, :], in_=st2[:, j, :])
        lnv2 = work.tile([128, 4], F32, name="lnv2")
        nc.scalar.activation(out=lnv2[:], in_=mv2[:, :, 1], func=AF.Ln,
                             bias=eps_l[:], scale=1.0)
        rstd2 = work.tile([128, 4], F32, name="rstd2")
        nc.scalar.activation(out=rstd2[:], in_=lnv2[:], func=AF.Exp, scale=-0.5)
        res_sb = work.tile([128, 4, 256], F32, name="res_sb")
        for j in range(4):
            g2r = work.tile([128, 256], F32, name="g2r")
            nc.vector.tensor_scalar_mul(g2r[:], g2rep[:], rstd2[:, j:j + 1])
            nc.vector.scalar_tensor_tensor(
                out=res_sb[:, j, :], in0=msg2_ps[:, j, :],
                scalar=mv2[:, j, 0:1], in1=g2r[:],
                op0=mybir.AluOpType.subtract, op1=mybir.AluOpType.mult)
        nc.gpsimd.dma_start(out=res_d[:, 4 * t:4 * (t + 1), :], in_=res_sb[:])


def _prep_host(inputs, TBLK, SBLK):
    """Shared host-side prep. Returns (const_map, per-core fn)."""
    bf = ml_dtypes.bfloat16
    Wq, Wk, Wv = inputs["Wq"], inputs["Wk"], inputs["Wv"]
    Wm, W1, W2 = inputs["Wmerge"], inputs["Wmlp1"], inputs["Wmlp2"]
    g1, b1 = inputs["ln1_g"], inputs["ln1_b"]
    g2 = inputs["ln2_g"]
    # fold ln1 gamma/beta into W1: h = relu(cat[x, g1*n + b1] @ W1.T)
    W1s = W1.copy()
    W1s[:, 256:] = W1[:, 256:] * g1[None, :]
    b1p = (W1[:, 256:] @ b1).astype(np.float32)          # [512]
    const = {
        "wq_t": np.ascontiguousarray(
            Wq.T.reshape(2, 128, 2, 128).transpose(1, 0, 2, 3)).astype(bf),
        "wk_r": np.ascontiguousarray(Wk.T.reshape(2, 128, 256)
                                     .transpose(1, 0, 2)).astype(bf),
        "wv_r": np.ascontiguousarray(Wv.T.reshape(2, 128, 256)
                                     .transpose(1, 0, 2)).astype(bf),
        "wm_r": np.ascontiguousarray(Wm.T.reshape(2, 128, 256)
                                     .transpose(1, 0, 2)).astype(bf),
        "w1_t": np.ascontiguousarray(
            W1s.T.reshape(4, 128, 4, 128).transpose(1, 0, 2, 3)).astype(bf),
        "w2_r": np.ascontiguousarray(W2.T.reshape(4, 128, 256)
                                     .transpose(1, 0, 2)).astype(bf),
        "b1p": np.ascontiguousarray(b1p.reshape(4, 128).T).astype(np.float32),
        "g2rep": np.broadcast_to(g2.astype(np.float32), (128, 256)).copy(),
        "ident": np.eye(128, dtype=bf),
    }

    def blocks(a, nblk):  # token-major [T, C] -> feature-major [128, 2, T] bf16
        del nblk
        T = a.shape[0]
        return np.ascontiguousarray(
            a.T.reshape(2, 128, T).transpose(1, 0, 2)).astype(bf)

    return const, blocks


TRACE = False        # set by test harness for NTFF profiling
LAST_RESULT = None   # BassKernelResults of the last kernel() call


def kernel(**inputs):
    global LAST_RESULT
    TBLK, SBLK = 32, 64
    N, L, C = inputs["x"].shape
    x = np.asarray(inputs["x"], np.float32)
    source = np.asarray(inputs["source"], np.float32)
    const, blocks = _prep_host(inputs, TBLK, SBLK)

    nc = bacc.Bacc("TRN2", target_bir_lowering=False, debug=False, num_devices=8)
    with tile.TileContext(nc) as tc:
        nc.tc = tc
        build_kernel(nc, TBLK, SBLK)
        nc._pool_ctx.close()
    nc.compile()

    in_maps = []
    for c in range(8):
        n, half = c // 2, c % 2
        xs = x[n, 4096 * half:4096 * (half + 1)]
        in_maps.append({**const,
                        "x_pre": blocks(xs, TBLK),
                        "s_pre": blocks(source[n], SBLK)})
    LAST_RESULT = run_bass_kernel_spmd(nc, in_maps, core_ids=list(range(8)),
                                       trace=TRACE)
    res = LAST_RESULT.results

    out = np.empty((N, L, C), np.float32)
    b2 = np.asarray(inputs["ln2_b"], np.float32)
    for c in range(8):
        n, half = c // 2, c % 2
        r = res[c]["res"].transpose(1, 0, 2).reshape(4096, C)
        out[n, 4096 * half:4096 * (half + 1)] = (
            x[n, 4096 * half:4096 * (half + 1)] + b2[None, :] + r)
    return out

